# revision 12
# baseline (speedup 1.0000x reference)
"""MMoE-style CustomizedGateControl kernel for 8x TRN2 NeuronCores.

Data-parallel over the batch dim (16384 -> 8 x 2048). Per core, everything
runs in the transposed [feature, batch] layout so the tower GEMMs need no
transpose and bias+ReLU fuse into the scalar-engine PSUM drain:
  - 12 expert GEMMs + gates as f16 matmuls with the weight chunk stationary
    and batch streaming (N=512), output [e, b] in PSUM
  - drain: ACT relu(psum + per-partition bias) -> f16 SBUF
  - gates [16, b] are broadcast to all 128 partitions via a DRAM round-trip
    DMA (stride-0 source), then the gated combine runs on the vector engine
    as f16 tensor_tensor mult/add into info[t] [e, b]
  - tower MLPs consume info [e, b] directly on the PE
All parameters replicated; no collectives.
"""

import sys

if "/opt/trn_rl_repo" not in sys.path:
    sys.path.insert(0, "/opt/trn_rl_repo")

import numpy as np

import concourse.bacc as bacc
import concourse.mybir as mybir
import concourse.tile as tile
from concourse.bass_utils import run_bass_kernel_spmd

# problem dims
B, D, E, H = 16384, 512, 256, 128
S, K, T = 4, 4, 2
NCORES = 8
BC = B // NCORES          # 2048 batch rows per core
P = 128                   # partitions
KC = D // P               # 4 contraction chunks
NE = S + T * K            # 12 experts
G = S + K                 # 8 gate inputs per task
EC = E // P               # 2 e-chunks per expert
JW = 512                  # batch columns per matmul / psum bank
NJ = BC // JW             # 4 batch blocks
WCOLS = NE * E            # 3072 expert weight columns
WALL = WCOLS + T * G      # 3088 = experts + gate columns

f32 = mybir.dt.float32
f16 = mybir.dt.float16

# expert order n: shared 0..3, task0 experts 4..7, task1 experts 8..11.
# tasks_of(n) -> list of (t, gate_index) pairs the expert feeds.
def _tasks_of(n):
    if n < S:
        return [(t, n) for t in range(T)]
    t = (n - S) // K
    return [(t, S + (n - S) % K)]


def _build():
    nc = bacc.Bacc("TRN2", target_bir_lowering=False, debug=False)

    xt_d = nc.dram_tensor("xt", [D, BC], f16, kind="ExternalInput").ap()
    wall_d = nc.dram_tensor("wall", [D, WALL], f16, kind="ExternalInput").ap()
    be_d = nc.dram_tensor("be", [P, NE * EC], f32, kind="ExternalInput").ap()
    tw1_d = nc.dram_tensor("tw1", [T, E, H], f16, kind="ExternalInput").ap()
    tb1_d = nc.dram_tensor("tb1", [H, T], f32, kind="ExternalInput").ap()
    tw2_d = nc.dram_tensor("tw2", [H, T], f16, kind="ExternalInput").ap()
    out_d = nc.dram_tensor("out", [T, BC], f32, kind="ExternalOutput").ap()

    with tile.TileContext(nc) as tc:
        with (
            tc.tile_pool(name="gdram", bufs=1, space="DRAM") as gdram_pool,
            tc.tile_pool(name="const", bufs=1) as const,
            tc.tile_pool(name="expt", bufs=6) as expt_pool,
            tc.tile_pool(name="prod", bufs=4) as prod_pool,
            tc.tile_pool(name="hsb", bufs=2) as hsb_pool,
        ):
            # ---- persistent inputs ----
            gd = gdram_pool.tile([T * G, BC], f16, tag="gd", name="gd")
            xt_t = [const.tile([P, BC], f16, tag=f"xt{k}", name=f"xt{k}") for k in range(KC)]
            wall_t = [const.tile([P, WALL], f16, tag=f"wall{k}", name=f"wall{k}") for k in range(KC)]
            be = const.tile([P, NE * EC], f32, tag="be", name="be")
            tb1 = const.tile([H, T], f32, tag="tb1", name="tb1")
            tw2 = const.tile([H, T], f16, tag="tw2", name="tw2")
            gsb = const.tile([T * G, BC], f16, tag="gsb", name="gsb")
            grep = {}
            for t in range(T):
                for g in range(G):
                    grep[(t, g)] = const.tile(
                        [P, BC], f16, tag=f"grep{t}_{g}", name=f"grep{t}_{g}"
                    )
            infoT = {}
            for t in range(T):
                for ec in range(EC):
                    infoT[(t, ec)] = const.tile(
                        [P, BC], f16, tag=f"infoT{t}_{ec}", name=f"infoT{t}_{ec}"
                    )
            tw1_t = {}
            for t in range(T):
                for ec in range(EC):
                    tw1_t[(t, ec)] = const.tile(
                        [P, H], f16, tag=f"tw1_{t}_{ec}", name=f"tw1_{t}_{ec}"
                    )
            out_sb = const.tile([1, T * BC], f32, tag="out_sb", name="out_sb")

            # input DMA: xt split halves on sync/vector queues (k-major so the
            # first expert group can start as chunks land); wall gate cols
            # first, then expert chunks n-major on gpsimd/scalar queues.
            for k in range(KC):
                rs = slice(k * P, (k + 1) * P)
                nc.sync.dma_start(xt_t[k][:, 0:1024], xt_d[rs, 0:1024])
                nc.sync.dma_start(xt_t[k][:, 1024:2048], xt_d[rs, 1024:2048])
            for k in range(KC):
                rs = slice(k * P, (k + 1) * P)
                nc.gpsimd.dma_start(wall_t[k][:, WCOLS:WALL], wall_d[rs, WCOLS:WALL])
            for n in range(NE):
                cs = slice(n * E, (n + 1) * E)
                for k in range(KC):
                    rs = slice(k * P, (k + 1) * P)
                    q = nc.gpsimd if (k % 2 == 0) else nc.scalar
                    q.dma_start(wall_t[k][:, cs], wall_d[rs, cs])
            nc.scalar.dma_start(be[:], be_d[:])
            for t in range(T):
                for ec in range(EC):
                    nc.scalar.dma_start(
                        tw1_t[(t, ec)][:], tw1_d[t, ec * P : (ec + 1) * P, :]
                    )
            nc.scalar.dma_start(tb1[:], tb1_d[:])
            nc.scalar.dma_start(tw2[:], tw2_d[:])

            with (
                tc.tile_pool(name="expps", bufs=4, space="PSUM") as expps_pool,
                tc.tile_pool(name="gaps", bufs=2, space="PSUM") as gaps_pool,
                tc.tile_pool(name="hps", bufs=2, space="PSUM") as hps_pool,
            ):
                # ---- gates: [16, b] psum -> f16 sbuf -> DRAM -> broadcast ----
                for j in range(NJ):
                    js = slice(j * JW, (j + 1) * JW)
                    gp = gaps_pool.tile([T * G, JW], f32, tag="gaps", name="gaps")
                    for k in range(KC):
                        nc.tensor.matmul(
                            gp[:],
                            wall_t[k][:, WCOLS:WALL],
                            xt_t[k][:, js],
                            start=(k == 0),
                            stop=(k == KC - 1),
                        )
                    nc.scalar.copy(gsb[:, js], gp[:])
                    nc.scalar.dma_start(gd[:, js], gsb[:, js])
                # broadcast each gate row to all 128 partitions, in combine
                # consumption order
                rows = [(t, g) for g in range(S) for t in range(T)]
                rows += [(0, g) for g in range(S, G)]
                rows += [(1, g) for g in range(S, G)]
                bqueues = [nc.sync, nc.gpsimd, nc.scalar]
                for i, (t, g) in enumerate(rows):
                    r = t * G + g
                    bqueues[i % 3].dma_start(
                        grep[(t, g)][:], gd[r : r + 1, :].broadcast_to([P, BC])
                    )

                # ---- expert sweep + incremental gated combine ----
                info_init = set()

                def emit_tower1(t):
                    for j in range(NJ):
                        js = slice(j * JW, (j + 1) * JW)
                        hp = hps_pool.tile([P, JW], f32, tag="hps", name="hps")
                        for ec in range(EC):
                            nc.tensor.matmul(
                                hp[:],
                                tw1_t[(t, ec)][:],
                                infoT[(t, ec)][:, js],
                                start=(ec == 0),
                                stop=(ec == EC - 1),
                            )
                        hs = hsb_pool.tile([P, JW], f16, tag="hsb", name="hsb")
                        nc.scalar.activation(
                            hs[:],
                            hp[:],
                            mybir.ActivationFunctionType.Relu,
                            bias=tb1[:, t : t + 1],
                        )
                        op = gaps_pool.tile([T * G, JW], f32, tag="gaps", name="gaps")
                        nc.tensor.matmul(
                            op[0:1, :], tw2[:, t : t + 1], hs[:], start=True, stop=True
                        )
                        r = t * NJ + j
                        nc.vector.tensor_copy(
                            out_sb[0:1, r * JW : (r + 1) * JW], op[0:1, :]
                        )
                        nc.sync.dma_start(
                            out_d.rearrange("t n -> (t n)")[
                                None, r * JW : (r + 1) * JW
                            ],
                            out_sb[0:1, r * JW : (r + 1) * JW],
                        )

                for n in range(NE):
                    for ec in range(EC):
                        c0 = n * E + ec * P
                        expT = expt_pool.tile([P, BC], f16, tag="expt", name="expt")
                        for j in range(NJ):
                            js = slice(j * JW, (j + 1) * JW)
                            pe = expps_pool.tile([P, JW], f32, tag="expps", name="expps")
                            for k in range(KC):
                                nc.tensor.matmul(
                                    pe[:],
                                    wall_t[k][:, c0 : c0 + P],
                                    xt_t[k][:, js],
                                    start=(k == 0),
                                    stop=(k == KC - 1),
                                )
                            nc.scalar.activation(
                                expT[:, js],
                                pe[:],
                                mybir.ActivationFunctionType.Relu,
                                bias=be[:, n * EC + ec : n * EC + ec + 1],
                            )
                        for t, g in _tasks_of(n):
                            if (t, ec) not in info_init:
                                info_init.add((t, ec))
                                nc.vector.tensor_mul(
                                    infoT[(t, ec)][:], expT[:], grep[(t, g)][:]
                                )
                            else:
                                pr = prod_pool.tile([P, BC], f16, tag="prod", name="prod")
                                nc.vector.tensor_mul(pr[:], expT[:], grep[(t, g)][:])
                                nc.vector.tensor_add(
                                    infoT[(t, ec)][:], infoT[(t, ec)][:], pr[:]
                                )
                    # interleave towers once their inputs are (nearly) ready
                    if n == 9:
                        emit_tower1(0)
                    if n == NE - 1:
                        emit_tower1(1)

    nc.compile()
    return nc


_NC = None


def _get_nc():
    global _NC
    if _NC is None:
        _NC = _build()
    return _NC


def _prep_shared(shared_W, shared_b, task_W, task_b, gate_W, tower_W1, tower_b1, tower_W2):
    cols = [np.asarray(shared_W[s]) for s in range(S)]
    cols += [np.asarray(task_W[t, k]) for t in range(T) for k in range(K)]
    cols += [np.asarray(gate_W[t]) for t in range(T)]  # col t*G+g = gate (t, g)
    wall = np.ascontiguousarray(np.concatenate(cols, axis=1), dtype=np.float16)
    bias_all = np.concatenate(
        [np.asarray(shared_b).reshape(-1), np.asarray(task_b).reshape(-1)]
    ).astype(np.float32)
    # be column n*EC+ec = bias of expert n, e-chunk ec, as a per-partition vec
    be = np.ascontiguousarray(
        bias_all.reshape(NE * EC, P).T.astype(np.float32)
    )
    tw1 = np.ascontiguousarray(tower_W1, dtype=np.float16)
    tb1 = np.ascontiguousarray(np.asarray(tower_b1).T, dtype=np.float32)   # [H, T]
    tw2 = np.ascontiguousarray(np.asarray(tower_W2)[:, :, 0].T, dtype=np.float16)  # [H, T]
    return wall, be, tw1, tb1, tw2


def kernel(
    x,
    shared_W,
    shared_b,
    task_W,
    task_b,
    gate_W,
    tower_W1,
    tower_b1,
    tower_W2,
    tower_b2,
    _trace=False,
    _tmpdir=None,
):
    nc = _get_nc()
    x = np.asarray(x, dtype=np.float32)
    wall, be, tw1, tb1, tw2 = _prep_shared(
        shared_W, shared_b, task_W, task_b, gate_W, tower_W1, tower_b1, tower_W2
    )
    in_maps = []
    for c in range(NCORES):
        xt = np.ascontiguousarray(x[c * BC : (c + 1) * BC, :].T.astype(np.float16))
        in_maps.append(
            {
                "xt": xt,
                "wall": wall,
                "be": be,
                "tw1": tw1,
                "tb1": tb1,
                "tw2": tw2,
            }
        )
    kw = {}
    if _trace:
        kw = {"trace": True, "tmpdir": _tmpdir}
    res = run_bass_kernel_spmd(nc, in_maps, core_ids=list(range(NCORES)), **kw)
    out = np.concatenate([res.results[c]["out"] for c in range(NCORES)], axis=1)
    out = out + np.asarray(tower_b2, dtype=np.float32)[:, 0][:, None]
    result = out[:, :, None].astype(np.float32)  # [T, B, 1]
    if _trace:
        return result, res
    return result


# revision 13
# speedup vs baseline: 1.2516x; 1.2516x over previous
"""MMoE-style CustomizedGateControl kernel for 8x TRN2 NeuronCores.

Data-parallel over the batch dim (16384 -> 8 x 2048). Per core, everything
runs in the transposed [feature, batch] layout so the tower GEMMs need no
transpose and bias+ReLU fuse into the scalar-engine PSUM drain:
  - 12 expert GEMMs + gates as f16 matmuls with the weight chunk stationary
    and batch streaming (N=512), output [e, b] in PSUM
  - drain: ACT relu(psum + per-partition bias) -> f16 SBUF, 1024 cols/op
  - gates [16, b] are broadcast to all 128 partitions via a DRAM round-trip
    DMA (stride-0 source), then the gated combine runs on the vector engine
    as f16 tensor_tensor mult/add into info[t] [e, b]
  - tower MLPs consume info [e, b] directly on the PE
All parameters replicated; no collectives.
"""

import sys

if "/opt/trn_rl_repo" not in sys.path:
    sys.path.insert(0, "/opt/trn_rl_repo")

import numpy as np

import concourse.bacc as bacc
import concourse.mybir as mybir
import concourse.tile as tile
from concourse.bass_utils import run_bass_kernel_spmd

# problem dims
B, D, E, H = 16384, 512, 256, 128
S, K, T = 4, 4, 2
NCORES = 8
BC = B // NCORES          # 2048 batch rows per core
P = 128                   # partitions
KC = D // P               # 4 contraction chunks
NE = S + T * K            # 12 experts
G = S + K                 # 8 gate inputs per task
EC = E // P               # 2 e-chunks per expert
JW = 512                  # batch columns per matmul
NJ = BC // JW             # 4 batch blocks
GOFF = T * G              # 16 gate cols, laid out FIRST in wall
WALL = GOFF + NE * E      # 3088

f32 = mybir.dt.float32
f16 = mybir.dt.float16


def _tasks_of(n):
    """Expert order n: shared 0..3, task0 4..7, task1 8..11 -> (t, gate_idx)."""
    if n < S:
        return [(t, n) for t in range(T)]
    t = (n - S) // K
    return [(t, S + (n - S) % K)]


def _build():
    nc = bacc.Bacc("TRN2", target_bir_lowering=False, debug=False)

    xt_d = nc.dram_tensor("xt", [D, BC], f16, kind="ExternalInput").ap()
    wall_d = nc.dram_tensor("wall", [D, WALL], f16, kind="ExternalInput").ap()
    be_d = nc.dram_tensor("be", [P, NE * EC], f32, kind="ExternalInput").ap()
    tw1_d = nc.dram_tensor("tw1", [T, E, H], f16, kind="ExternalInput").ap()
    tb1_d = nc.dram_tensor("tb1", [H, T], f32, kind="ExternalInput").ap()
    tw2_d = nc.dram_tensor("tw2", [H, T], f16, kind="ExternalInput").ap()
    out_d = nc.dram_tensor("out", [T, BC], f32, kind="ExternalOutput").ap()

    with tile.TileContext(nc) as tc:
        with (
            tc.tile_pool(name="gdram", bufs=1, space="DRAM") as gdram_pool,
            tc.tile_pool(name="const", bufs=1) as const,
            tc.tile_pool(name="expt", bufs=6) as expt_pool,
            tc.tile_pool(name="prod", bufs=4) as prod_pool,
            tc.tile_pool(name="hsb", bufs=2) as hsb_pool,
        ):
            gd = gdram_pool.tile([T * G, BC], f16, tag="gd", name="gd")
            xt_t = [const.tile([P, BC], f16, tag=f"xt{k}", name=f"xt{k}") for k in range(KC)]
            wall_t = [const.tile([P, WALL], f16, tag=f"wall{k}", name=f"wall{k}") for k in range(KC)]
            be = const.tile([P, NE * EC], f32, tag="be", name="be")
            tb1 = const.tile([H, T], f32, tag="tb1", name="tb1")
            tw2 = const.tile([H, T], f16, tag="tw2", name="tw2")
            gsb = const.tile([T * G, BC], f16, tag="gsb", name="gsb")
            grep = {}
            for t in range(T):
                for g in range(G):
                    grep[(t, g)] = const.tile(
                        [P, BC], f16, tag=f"grep{t}_{g}", name=f"grep{t}_{g}"
                    )
            infoT = {}
            for t in range(T):
                for ec in range(EC):
                    infoT[(t, ec)] = const.tile(
                        [P, BC], f16, tag=f"infoT{t}_{ec}", name=f"infoT{t}_{ec}"
                    )
            tw1_t = {}
            for t in range(T):
                for ec in range(EC):
                    tw1_t[(t, ec)] = const.tile(
                        [P, H], f16, tag=f"tw1_{t}_{ec}", name=f"tw1_{t}_{ec}"
                    )
            out_sb = const.tile([1, T * BC], f32, tag="out_sb", name="out_sb")

            # ---- input DMA: few big transfers; scalar queue kept clear for
            # ACT drains (only tiny early const loads go there) ----
            # sync: be, then xt (k-major, halves so early j-blocks land first)
            nc.sync.dma_start(be[:], be_d[:])
            for k in range(KC):
                rs = slice(k * P, (k + 1) * P)
                nc.sync.dma_start(xt_t[k][:, 0:1024], xt_d[rs, 0:1024])
            for k in range(KC):
                rs = slice(k * P, (k + 1) * P)
                nc.sync.dma_start(xt_t[k][:, 1024:2048], xt_d[rs, 1024:2048])
            # gpsimd: wall in 3 column groups: [gates+shared | task0 | task1]
            WA = GOFF + S * E          # 1040
            for k in range(KC):
                rs = slice(k * P, (k + 1) * P)
                nc.gpsimd.dma_start(wall_t[k][:, 0:WA], wall_d[rs, 0:WA])
            for k in range(KC):
                rs = slice(k * P, (k + 1) * P)
                nc.gpsimd.dma_start(wall_t[k][:, WA : WA + K * E], wall_d[rs, WA : WA + K * E])
            for k in range(KC):
                rs = slice(k * P, (k + 1) * P)
                nc.gpsimd.dma_start(wall_t[k][:, WA + K * E : WALL], wall_d[rs, WA + K * E : WALL])
            # scalar: tiny consts, all early, done before first drains matter
            for t in range(T):
                for ec in range(EC):
                    nc.scalar.dma_start(
                        tw1_t[(t, ec)][:], tw1_d[t, ec * P : (ec + 1) * P, :]
                    )
            nc.scalar.dma_start(tb1[:], tb1_d[:])
            nc.scalar.dma_start(tw2[:], tw2_d[:])

            with (
                tc.tile_pool(name="expps", bufs=2, space="PSUM") as expps_pool,
                tc.tile_pool(name="hps", bufs=2, space="PSUM") as hps_pool,
            ):
                # ---- gates: [16, b] psum -> f16 sbuf -> DRAM -> broadcast ----
                for j in range(NJ):
                    js = slice(j * JW, (j + 1) * JW)
                    gp = hps_pool.tile([T * G, JW], f32, tag="hps", name="gps")
                    for k in range(KC):
                        nc.tensor.matmul(
                            gp[:],
                            wall_t[k][:, 0:GOFF],
                            xt_t[k][:, js],
                            start=(k == 0),
                            stop=(k == KC - 1),
                        )
                    nc.scalar.copy(gsb[:, js], gp[:])
                nc.sync.dma_start(gd[:], gsb[:])
                # broadcast each gate row to all 128 partitions, in combine
                # consumption order, split across sync/gpsimd queues
                rows = [(t, g) for g in range(S) for t in range(T)]
                rows += [(0, g) for g in range(S, G)]
                rows += [(1, g) for g in range(S, G)]
                bqueues = [nc.sync, nc.gpsimd]
                for i, (t, g) in enumerate(rows):
                    r = t * G + g
                    bqueues[i % 2].dma_start(
                        grep[(t, g)][:], gd[r : r + 1, :].broadcast_to([P, BC])
                    )

                # ---- expert sweep + incremental gated combine ----
                info_init = set()

                def emit_tower(t):
                    for jp in range(NJ // 2):
                        hp = hps_pool.tile([P, 2 * JW], f32, tag="hps", name="hps")
                        for j2 in range(2):
                            js = slice((jp * 2 + j2) * JW, (jp * 2 + j2 + 1) * JW)
                            for ec in range(EC):
                                nc.tensor.matmul(
                                    hp[:, j2 * JW : (j2 + 1) * JW],
                                    tw1_t[(t, ec)][:],
                                    infoT[(t, ec)][:, js],
                                    start=(ec == 0),
                                    stop=(ec == EC - 1),
                                )
                        hs = hsb_pool.tile([P, 2 * JW], f16, tag="hsb", name="hsb")
                        nc.scalar.activation(
                            hs[:],
                            hp[:],
                            mybir.ActivationFunctionType.Relu,
                            bias=tb1[:, t : t + 1],
                        )
                        op = hps_pool.tile([T * G, 2 * JW], f32, tag="hps", name="ops")
                        for j2 in range(2):
                            nc.tensor.matmul(
                                op[0:1, j2 * JW : (j2 + 1) * JW],
                                tw2[:, t : t + 1],
                                hs[:, j2 * JW : (j2 + 1) * JW],
                                start=True,
                                stop=True,
                            )
                        r0 = (t * NJ + jp * 2) * JW
                        nc.vector.tensor_copy(
                            out_sb[0:1, r0 : r0 + 2 * JW], op[0:1, :]
                        )
                        nc.sync.dma_start(
                            out_d.rearrange("t n -> (t n)")[None, r0 : r0 + 2 * JW],
                            out_sb[0:1, r0 : r0 + 2 * JW],
                        )

                for n in range(NE):
                    for ec in range(EC):
                        c0 = GOFF + n * E + ec * P
                        expT = expt_pool.tile([P, BC], f16, tag="expt", name="expt")
                        for jp in range(NJ // 2):
                            pe = expps_pool.tile([P, 2 * JW], f32, tag="expps", name="expps")
                            for j2 in range(2):
                                js = slice((jp * 2 + j2) * JW, (jp * 2 + j2 + 1) * JW)
                                for k in range(KC):
                                    nc.tensor.matmul(
                                        pe[:, j2 * JW : (j2 + 1) * JW],
                                        wall_t[k][:, c0 : c0 + P],
                                        xt_t[k][:, js],
                                        start=(k == 0),
                                        stop=(k == KC - 1),
                                    )
                            nc.scalar.activation(
                                expT[:, jp * 2 * JW : (jp + 1) * 2 * JW],
                                pe[:],
                                mybir.ActivationFunctionType.Relu,
                                bias=be[:, n * EC + ec : n * EC + ec + 1],
                            )
                        for t, g in _tasks_of(n):
                            if (t, ec) not in info_init:
                                info_init.add((t, ec))
                                nc.vector.tensor_mul(
                                    infoT[(t, ec)][:], expT[:], grep[(t, g)][:]
                                )
                            else:
                                pr = prod_pool.tile([P, BC], f16, tag="prod", name="prod")
                                nc.vector.tensor_mul(pr[:], expT[:], grep[(t, g)][:])
                                nc.vector.tensor_add(
                                    infoT[(t, ec)][:], infoT[(t, ec)][:], pr[:]
                                )
                    if n == 9:
                        emit_tower(0)
                    if n == NE - 1:
                        emit_tower(1)

    nc.compile()
    return nc


_NC = None


def _get_nc():
    global _NC
    if _NC is None:
        _NC = _build()
    return _NC


def _prep_shared(shared_W, shared_b, task_W, task_b, gate_W, tower_W1, tower_b1, tower_W2):
    cols = [np.asarray(gate_W[t]) for t in range(T)]  # gate col t*G+g first
    cols += [np.asarray(shared_W[s]) for s in range(S)]
    cols += [np.asarray(task_W[t, k]) for t in range(T) for k in range(K)]
    wall = np.ascontiguousarray(np.concatenate(cols, axis=1), dtype=np.float16)
    bias_all = np.concatenate(
        [np.asarray(shared_b).reshape(-1), np.asarray(task_b).reshape(-1)]
    ).astype(np.float32)
    # be column n*EC+ec = bias of expert n, e-chunk ec, as a per-partition vec
    be = np.ascontiguousarray(bias_all.reshape(NE * EC, P).T.astype(np.float32))
    tw1 = np.ascontiguousarray(tower_W1, dtype=np.float16)
    tb1 = np.ascontiguousarray(np.asarray(tower_b1).T, dtype=np.float32)   # [H, T]
    tw2 = np.ascontiguousarray(np.asarray(tower_W2)[:, :, 0].T, dtype=np.float16)  # [H, T]
    return wall, be, tw1, tb1, tw2


def kernel(
    x,
    shared_W,
    shared_b,
    task_W,
    task_b,
    gate_W,
    tower_W1,
    tower_b1,
    tower_W2,
    tower_b2,
    _trace=False,
    _tmpdir=None,
):
    nc = _get_nc()
    x = np.asarray(x, dtype=np.float32)
    wall, be, tw1, tb1, tw2 = _prep_shared(
        shared_W, shared_b, task_W, task_b, gate_W, tower_W1, tower_b1, tower_W2
    )
    in_maps = []
    for c in range(NCORES):
        xt = np.ascontiguousarray(x[c * BC : (c + 1) * BC, :].T.astype(np.float16))
        in_maps.append(
            {
                "xt": xt,
                "wall": wall,
                "be": be,
                "tw1": tw1,
                "tb1": tb1,
                "tw2": tw2,
            }
        )
    kw = {}
    if _trace:
        kw = {"trace": True, "tmpdir": _tmpdir}
    res = run_bass_kernel_spmd(nc, in_maps, core_ids=list(range(NCORES)), **kw)
    out = np.concatenate([res.results[c]["out"] for c in range(NCORES)], axis=1)
    out = out + np.asarray(tower_b2, dtype=np.float32)[:, 0][:, None]
    result = out[:, :, None].astype(np.float32)  # [T, B, 1]
    if _trace:
        return result, res
    return result


# revision 19
# speedup vs baseline: 1.2775x; 1.0207x over previous
"""MMoE-style CustomizedGateControl kernel for 8x TRN2 NeuronCores.

Data-parallel over the batch dim (16384 -> 8 x 2048). Per core, everything
runs in the transposed [feature, batch] layout so the tower GEMMs need no
transpose and bias+ReLU fuse into the scalar-engine PSUM drain:
  - 12 expert GEMMs + gates as f16 matmuls with the weight chunk stationary
    and batch streaming (N=512), output [e, b] in PSUM
  - drain: ACT relu(psum + per-partition bias) -> f16 SBUF, 1024 cols/op
  - gates [16, b] are broadcast to all 128 partitions via a DRAM round-trip
    DMA (stride-0 source), then the gated combine runs on the vector engine
    as f16 tensor_tensor mult/add into info[t] [e, b]
  - tower MLPs consume info [e, b] directly on the PE
All parameters replicated; no collectives.
"""

import sys

if "/opt/trn_rl_repo" not in sys.path:
    sys.path.insert(0, "/opt/trn_rl_repo")

import numpy as np

import concourse.bacc as bacc
import concourse.mybir as mybir
import concourse.tile as tile
from concourse.bass_utils import run_bass_kernel_spmd

# problem dims
B, D, E, H = 16384, 512, 256, 128
S, K, T = 4, 4, 2
NCORES = 8
BC = B // NCORES          # 2048 batch rows per core
P = 128                   # partitions
KC = D // P               # 4 contraction chunks
NE = S + T * K            # 12 experts
G = S + K                 # 8 gate inputs per task
EC = E // P               # 2 e-chunks per expert
JW = 512                  # batch columns per matmul
NJ = BC // JW             # 4 batch blocks
GOFF = T * G              # 16 gate cols, laid out FIRST in wall
WALL = GOFF + NE * E      # 3088

f32 = mybir.dt.float32
f16 = mybir.dt.float16


def _tasks_of(n):
    """Expert order n: shared 0..3, task0 4..7, task1 8..11 -> (t, gate_idx)."""
    if n < S:
        return [(t, n) for t in range(T)]
    t = (n - S) // K
    return [(t, S + (n - S) % K)]


def _build():
    nc = bacc.Bacc("TRN2", target_bir_lowering=False, debug=False)

    xt_d = nc.dram_tensor("xt", [D, BC], f16, kind="ExternalInput").ap()
    wall_d = nc.dram_tensor("wall", [D, WALL], f16, kind="ExternalInput").ap()
    be_d = nc.dram_tensor("be", [P, NE * EC], f32, kind="ExternalInput").ap()
    tw1_d = nc.dram_tensor("tw1", [T, E, H], f16, kind="ExternalInput").ap()
    tb1_d = nc.dram_tensor("tb1", [H, T], f32, kind="ExternalInput").ap()
    tw2_d = nc.dram_tensor("tw2", [H, T], f16, kind="ExternalInput").ap()
    out_d = nc.dram_tensor("out", [T, BC], f32, kind="ExternalOutput").ap()

    with tile.TileContext(nc) as tc:
        with (
            tc.tile_pool(name="gdram", bufs=1, space="DRAM") as gdram_pool,
            tc.tile_pool(name="const", bufs=1) as const,
            tc.tile_pool(name="expt", bufs=6) as expt_pool,
            tc.tile_pool(name="prod", bufs=4) as prod_pool,
            tc.tile_pool(name="hsb", bufs=2) as hsb_pool,
        ):
            gd = gdram_pool.tile([T * G, BC], f16, tag="gd", name="gd")
            xt_t = [const.tile([P, BC], f16, tag=f"xt{k}", name=f"xt{k}") for k in range(KC)]
            wall_t = [const.tile([P, WALL], f16, tag=f"wall{k}", name=f"wall{k}") for k in range(KC)]
            be = const.tile([P, NE * EC], f32, tag="be", name="be")
            tb1 = const.tile([H, T], f32, tag="tb1", name="tb1")
            tw2 = const.tile([H, T], f16, tag="tw2", name="tw2")
            gsb = const.tile([T * G, BC], f16, tag="gsb", name="gsb")
            grep = {}
            for t in range(T):
                for g in range(G):
                    grep[(t, g)] = const.tile(
                        [P, BC], f16, tag=f"grep{t}_{g}", name=f"grep{t}_{g}"
                    )
            out_sb = const.tile([1, T * BC], f32, tag="out_sb", name="out_sb")
            infoT = {}
            for t in range(T):
                for ec in range(EC):
                    infoT[(t, ec)] = const.tile(
                        [P, BC], f16, tag=f"infoT{t}_{ec}", name=f"infoT{t}_{ec}"
                    )
            tw1_t = {}
            for t in range(T):
                for ec in range(EC):
                    tw1_t[(t, ec)] = const.tile(
                        [P, H], f16, tag=f"tw1_{t}_{ec}", name=f"tw1_{t}_{ec}"
                    )

            # ---- input DMA: few big transfers; xt halves split across the
            # sync and scalar queues so the full xt lands ASAP ----
            for k in range(KC):
                rs = slice(k * P, (k + 1) * P)
                nc.sync.dma_start(xt_t[k][:, 0:1024], xt_d[rs, 0:1024])
                nc.scalar.dma_start(xt_t[k][:, 1024:2048], xt_d[rs, 1024:2048])
            nc.sync.dma_start(be[:], be_d[:])
            # gpsimd: wall in 3 column groups: [gates+shared | task0 | task1]
            WA = GOFF + S * E          # 1040
            for k in range(KC):
                rs = slice(k * P, (k + 1) * P)
                nc.gpsimd.dma_start(wall_t[k][:, 0:WA], wall_d[rs, 0:WA])
            for k in range(KC):
                rs = slice(k * P, (k + 1) * P)
                nc.gpsimd.dma_start(wall_t[k][:, WA : WA + K * E], wall_d[rs, WA : WA + K * E])
            for k in range(KC):
                rs = slice(k * P, (k + 1) * P)
                nc.gpsimd.dma_start(wall_t[k][:, WA + K * E : WALL], wall_d[rs, WA + K * E : WALL])
            # scalar: tiny tower consts after its xt half, before drains start
            for t in range(T):
                for ec in range(EC):
                    nc.scalar.dma_start(
                        tw1_t[(t, ec)][:], tw1_d[t, ec * P : (ec + 1) * P, :]
                    )
            nc.scalar.dma_start(tb1[:], tb1_d[:])
            nc.scalar.dma_start(tw2[:], tw2_d[:])

            with (
                tc.tile_pool(name="expps", bufs=3, space="PSUM") as expps_pool,
                tc.tile_pool(name="hps", bufs=2, space="PSUM") as hps_pool,
            ):
                # ---- gates: [16, b] psum -> f16 sbuf -> DRAM -> broadcast ----
                for j in range(NJ):
                    js = slice(j * JW, (j + 1) * JW)
                    gp = hps_pool.tile([T * G, JW], f32, tag="hps", name="gps")
                    for k in range(KC):
                        nc.tensor.matmul(
                            gp[:],
                            wall_t[k][:, 0:GOFF],
                            xt_t[k][:, js],
                            start=(k == 0),
                            stop=(k == KC - 1),
                        )
                    nc.scalar.copy(gsb[:, js], gp[:])
                nc.sync.dma_start(gd[:], gsb[:])
                # broadcast each gate row to all 128 partitions. sync carries
                # the time-critical shared-expert rows in consumption order;
                # gpsimd (busy with wall until ~16us) carries task rows,
                # which aren't consumed until much later.
                for g in range(S):
                    for t in range(T):
                        nc.sync.dma_start(
                            grep[(t, g)][:],
                            gd[t * G + g : t * G + g + 1, :].broadcast_to([P, BC]),
                        )
                for t in range(T):
                    for g in range(S, G):
                        nc.gpsimd.dma_start(
                            grep[(t, g)][:],
                            gd[t * G + g : t * G + g + 1, :].broadcast_to([P, BC]),
                        )

                # ---- expert sweep + incremental gated combine ----
                info_init = set()

                def emit_tower(t):
                    for j in range(NJ):
                        js = slice(j * JW, (j + 1) * JW)
                        hp = hps_pool.tile([P, JW], f32, tag="hps", name="hps")
                        for ec in range(EC):
                            nc.tensor.matmul(
                                hp[:],
                                tw1_t[(t, ec)][:],
                                infoT[(t, ec)][:, js],
                                start=(ec == 0),
                                stop=(ec == EC - 1),
                            )
                        hs = hsb_pool.tile([P, JW], f16, tag="hsb", name="hsb")
                        nc.scalar.activation(
                            hs[:],
                            hp[:],
                            mybir.ActivationFunctionType.Relu,
                            bias=tb1[:, t : t + 1],
                        )
                        op = hps_pool.tile([T * G, JW], f32, tag="hps", name="ops")
                        nc.tensor.matmul(
                            op[0:1, :], tw2[:, t : t + 1], hs[:], start=True, stop=True
                        )
                        r0 = (t * NJ + j) * JW
                        nc.scalar.copy(out_sb[0:1, r0 : r0 + JW], op[0:1, :])
                        nc.sync.dma_start(
                            out_d.rearrange("t n -> (t n)")[None, r0 : r0 + JW],
                            out_sb[0:1, r0 : r0 + JW],
                        )

                for n in range(NE):
                    for ec in range(EC):
                        c0 = GOFF + n * E + ec * P
                        expT = expt_pool.tile([P, BC], f16, tag="expt", name="expt")
                        for jp in range(NJ // 2):
                            pe = expps_pool.tile([P, 2 * JW], f32, tag="expps", name="expps")
                            for j2 in range(2):
                                js = slice((jp * 2 + j2) * JW, (jp * 2 + j2 + 1) * JW)
                                for k in range(KC):
                                    nc.tensor.matmul(
                                        pe[:, j2 * JW : (j2 + 1) * JW],
                                        wall_t[k][:, c0 : c0 + P],
                                        xt_t[k][:, js],
                                        start=(k == 0),
                                        stop=(k == KC - 1),
                                    )
                            nc.scalar.activation(
                                expT[:, jp * 2 * JW : (jp + 1) * 2 * JW],
                                pe[:],
                                mybir.ActivationFunctionType.Relu,
                                bias=be[:, n * EC + ec : n * EC + ec + 1],
                            )
                        for t, g in _tasks_of(n):
                            if (t, ec) not in info_init:
                                info_init.add((t, ec))
                                nc.vector.tensor_mul(
                                    infoT[(t, ec)][:], expT[:], grep[(t, g)][:]
                                )
                            else:
                                pr = prod_pool.tile([P, BC], f16, tag="prod", name="prod")
                                nc.vector.tensor_mul(pr[:], expT[:], grep[(t, g)][:])
                                nc.vector.tensor_add(
                                    infoT[(t, ec)][:], infoT[(t, ec)][:], pr[:]
                                )
                    if n == 9:
                        emit_tower(0)
                    if n == NE - 1:
                        emit_tower(1)

    nc.compile()
    return nc


_NC = None


def _get_nc():
    global _NC
    if _NC is None:
        _NC = _build()
    return _NC


def _prep_shared(shared_W, shared_b, task_W, task_b, gate_W, tower_W1, tower_b1, tower_W2):
    cols = [np.asarray(gate_W[t]) for t in range(T)]  # gate col t*G+g first
    cols += [np.asarray(shared_W[s]) for s in range(S)]
    cols += [np.asarray(task_W[t, k]) for t in range(T) for k in range(K)]
    wall = np.ascontiguousarray(np.concatenate(cols, axis=1), dtype=np.float16)
    bias_all = np.concatenate(
        [np.asarray(shared_b).reshape(-1), np.asarray(task_b).reshape(-1)]
    ).astype(np.float32)
    # be column n*EC+ec = bias of expert n, e-chunk ec, as a per-partition vec
    be = np.ascontiguousarray(bias_all.reshape(NE * EC, P).T.astype(np.float32))
    tw1 = np.ascontiguousarray(tower_W1, dtype=np.float16)
    tb1 = np.ascontiguousarray(np.asarray(tower_b1).T, dtype=np.float32)   # [H, T]
    tw2 = np.ascontiguousarray(np.asarray(tower_W2)[:, :, 0].T, dtype=np.float16)  # [H, T]
    return wall, be, tw1, tb1, tw2


def kernel(
    x,
    shared_W,
    shared_b,
    task_W,
    task_b,
    gate_W,
    tower_W1,
    tower_b1,
    tower_W2,
    tower_b2,
    _trace=False,
    _tmpdir=None,
):
    nc = _get_nc()
    x = np.asarray(x, dtype=np.float32)
    wall, be, tw1, tb1, tw2 = _prep_shared(
        shared_W, shared_b, task_W, task_b, gate_W, tower_W1, tower_b1, tower_W2
    )
    in_maps = []
    for c in range(NCORES):
        xt = np.ascontiguousarray(x[c * BC : (c + 1) * BC, :].T.astype(np.float16))
        in_maps.append(
            {
                "xt": xt,
                "wall": wall,
                "be": be,
                "tw1": tw1,
                "tb1": tb1,
                "tw2": tw2,
            }
        )
    kw = {}
    if _trace:
        kw = {"trace": True, "tmpdir": _tmpdir}
    res = run_bass_kernel_spmd(nc, in_maps, core_ids=list(range(NCORES)), **kw)
    out = np.concatenate([res.results[c]["out"] for c in range(NCORES)], axis=1)
    out = out + np.asarray(tower_b2, dtype=np.float32)[:, 0][:, None]
    result = out[:, :, None].astype(np.float32)  # [T, B, 1]
    if _trace:
        return result, res
    return result
